# revision 20
# baseline (speedup 1.0000x reference)
"""Trainium2 Bass kernel for nn_ASSM_Illumination (B=1, L=65536, DIM=192, 8 cores).

Mathematical reduction
----------------------
The reference computes: convs -> routing MLP -> gumbel one-hot -> sort by
illumination key -> gated selective scan -> LayerNorm -> projection -> unsort.

The scan output is y[l] = (h_l @ A_log) + xs[l], where (h_l @ A_log) is a
per-token *scalar* broadcast over channels, and xs[l] = gamma_l * x[l] + beta_l
with per-token scalars gamma_l, beta_l.  The LayerNorm over channels is
invariant to per-token additive shifts, so the scan scalar and beta cancel
exactly; gamma cancels except through the eps term:

    LN(y)[l] = (x_l - mean(x_l)) / sqrt(var(x_l) + eps/gamma_l^2)

gamma_l = 0.3 + 0.7*sigmoid(key_l) in [0.65, 0.81], and with eps = 1e-5 the
output's sensitivity to gamma is ~1e-5 relative, far below the reference's own
fp32 noise floor.  The sort + unsort is a permutation and its inverse applied
around per-token ops: identity.

So the kernel computes, per token:
    out[l] = ((x_l - mu_l) * rstd_l) @ W + c
with W = (out_w * ln_w).T, c = ln_b @ out_w.T + out_b (c == 0 for this
problem's zero biases), rstd_l = 1/sqrt(var(x_l) + 1e-5/g0^2), g0 = 0.735.

Device dataflow (v5) -- per 128-token tile, fp16 matmul path:
  (x - mu)*rstd @ W = (rstd*x) @ W - (rstd*mu) * s   with s = colsum(W):
  - DVE:  paired-tile bn_stats on raw f32 x: two tiles streamed
          channel-interleaved (p a c -> p c a) land on the even/odd element
          stats of ONE bn_stats -> exact (mean, n*var) per tile, no bn_aggr
  - ACT:  rstd' = 1/sqrt(n*var + n*eps) batched (sqrt(n) folded into host
          weights); DVE reciprocal
  - DVE:  raw mu written into column 192 of the f32 input tile, so the
          per-tile cast+scale (xh = rstd * x, f32->f16, split DVE/ACT)
          produces the q = mu*rstd column for free
  - PE:   transpose [tok, 0:128] and [tok, 128:193], 8 tiles per full
          PSUM bank; one PSUM->SBUF copy per bank (DVE lo / ACT hi)
  - PE:   2 matmuls vs weights [W_lo] and [W_hi; -s] into f16 PSUM
  - DVE/ACT: plain 4-tile-batched f16->f32 output copies (scale already
          applied on the input side), alternating engines
  - DMA:  16-tile chunks; loads on SP hwdge, stores on Pool SWDGE

Sharding: L=65536 tokens split contiguously across 8 cores (8192 each); the
tiny weight matrix is replicated.  No collectives.  Token t of a shard lives
at partition p = t // 64, slot a = t % 64 so DMAs move long contiguous lines
per partition.
"""

import numpy as np
from contextlib import ExitStack

import concourse.bass as bass
import concourse.bacc as bacc
import concourse.tile as tile
from concourse import mybir
from concourse.masks import make_identity

L = 65536
DIM = 192
NCORES = 8
SHARD = L // NCORES          # 8192 tokens per core
P = 128                      # tokens per tile (partition dim)
G0 = 0.735                   # mid-range gamma; output sensitivity to g0 is ~1e-5
EPS_EFF = 1e-5 / (G0 * G0)
XW = DIM + 1                 # row: 192 channels + mu (f32) / q (f16)

F32 = mybir.dt.float32
F16 = mybir.dt.float16
AF = mybir.ActivationFunctionType
ALU = mybir.AluOpType


def bn_stats_raw(eng, out, in_):
    """Raw InstBNStats: bass's wrapper would treat the 3D input as groups,
    but the hardware streams the AP flat and computes separate even/odd
    element stats -- exactly the 2-tile interleave we want."""
    return eng.add_instruction(mybir.InstBNStats(
        name=eng.bass.get_next_instruction_name(),
        ins=[eng.lower_ap(in_)],
        outs=[eng.lower_ap(out)]))


def build_nc(shard=SHARD, chunk_tiles=16, group=8, quad=4):
    """One-core program; run SPMD on 8 cores with different x shards."""
    ntiles = shard // P          # 64
    nchunks = ntiles // chunk_tiles
    nc = bacc.Bacc("TRN2", target_bir_lowering=False, debug=False,
                   num_devices=NCORES)

    x_d = nc.dram_tensor("x_shard", (shard, DIM), F32, kind="ExternalInput")
    # wt_lo: sqrt(192)*W[0:128]  (f16)
    wlo_d = nc.dram_tensor("wt_lo", (128, DIM), F16, kind="ExternalInput")
    # wt_hi rows: [sqrt(192)*W[128:192]; -colsum(sqrt(192)*W)] -> [65, 192]
    whi_d = nc.dram_tensor("wt_hi", (65, DIM), F16, kind="ExternalInput")
    o_d = nc.dram_tensor("out_shard", (shard, DIM), F32, kind="ExternalOutput")

    # token t = p * (shard/128) + a  lives at partition p, slot a
    x3 = x_d[:, :].rearrange("(p a) c -> p a c", p=P)
    o3 = o_d[:, :].rearrange("(p a) c -> p a c", p=P)

    with tile.TileContext(nc) as tc, ExitStack() as ctx:
        singles = ctx.enter_context(tc.tile_pool(name="singles", bufs=1))
        xin = ctx.enter_context(tc.tile_pool(name="xin", bufs=3))
        xh_pool = ctx.enter_context(tc.tile_pool(name="xh", bufs=3))
        xout = ctx.enter_context(tc.tile_pool(name="xout", bufs=3))
        stats = ctx.enter_context(tc.tile_pool(name="stats", bufs=4))
        sbT = ctx.enter_context(tc.tile_pool(name="sbT", bufs=3))
        ps_lo = ctx.enter_context(
            tc.tile_pool(name="ps_lo", bufs=2, space=bass.MemorySpace.PSUM))
        ps_hi = ctx.enter_context(
            tc.tile_pool(name="ps_hi", bufs=2, space=bass.MemorySpace.PSUM))
        ps_z = ctx.enter_context(
            tc.tile_pool(name="ps_z", bufs=2, space=bass.MemorySpace.PSUM))

        ident = singles.tile([P, P], F16)
        make_identity(nc, ident)
        eps_t = singles.tile([P, 1], F32)
        nc.vector.memset(eps_t, float(DIM * EPS_EFF))
        # weights ride the ACT hwdge queue so SP's first trigger is the
        # first x-chunk load
        wlo = singles.tile([128, DIM], F16)
        nc.scalar.dma_start(out=wlo, in_=wlo_d[:, :])
        whi = singles.tile([65, DIM], F16)
        nc.scalar.dma_start(out=whi, in_=whi_d[:, :])

        ngroups = chunk_tiles // group

        def front(n):
            """load + stats + cast for chunk n; returns the f16 tile."""
            a0 = n * chunk_tiles
            xc = xin.tile([P, chunk_tiles, XW], F32)
            if n == 0:
                # split the first load so the stats pipeline starts sooner
                for (b0, b1) in ((0, 2), (2, 4), (4, 8), (8, chunk_tiles)):
                    nc.sync.dma_start(out=xc[:, b0:b1, 0:DIM],
                                      in_=x3[:, a0 + b0:a0 + b1, :])
            else:
                nc.sync.dma_start(out=xc[:, :, 0:DIM],
                                  in_=x3[:, a0:a0 + chunk_tiles, :])

            st = stats.tile([P, chunk_tiles // 2, 6], F32, tag="st")
            stdv = stats.tile([P, chunk_tiles], F32, tag="stdv")
            rstd = stats.tile([P, chunk_tiles], F32, tag="rstd")
            xh = xh_pool.tile([P, chunk_tiles, XW], F16)
            H = chunk_tiles // 2
            for h in range(2):
                h0 = h * H
                # DVE: paired-tile stats (even/odd split of one bn_stats)
                for k in range(h0, h0 + H, 2):
                    pair = xc[:, k:k + 2, 0:DIM].rearrange("p a c -> p c a")
                    bn_stats_raw(nc.vector, st[:, k // 2, :], pair)
                # rstd' = 1/sqrt(n*var + n*eps); raw mu -> xc col 192
                sh = st[:, h0 // 2:(h0 + H) // 2, :]
                nv = sh[:, :, 2:6:3]   # strided view of n*var
                mu = sh[:, :, 1:5:3]   # strided view of means
                nc.scalar.activation(out=stdv[:, h0:h0 + H], in_=nv,
                                     func=AF.Sqrt, bias=eps_t)
                nc.vector.reciprocal(out=rstd[:, h0:h0 + H],
                                     in_=stdv[:, h0:h0 + H])
                nc.vector.tensor_copy(out=xc[:, h0:h0 + H, DIM], in_=mu)
                # cast+scale: xh = rstd * [x | mu] (f16); col 192 becomes q
                for k in range(h0, h0 + H):
                    if k % 2 == 0:
                        nc.vector.tensor_scalar_mul(out=xh[:, k, :],
                                                    in0=xc[:, k, :],
                                                    scalar1=rstd[:, k:k + 1])
                    else:
                        nc.scalar.mul(out=xh[:, k, :], in_=xc[:, k, :],
                                      mul=rstd[:, k:k + 1])
            return xh

        def back(n, xh):
            """transpose + matmul + output for chunk n."""
            a0 = n * chunk_tiles
            oc = xout.tile([P, chunk_tiles, DIM], F32)
            for g in range(ngroups):
                g0 = g * group
                # PE transposes into full-bank PSUM tiles (8 tiles per bank)
                comb_lo = ps_lo.tile([128, group, P], F16)
                comb_hi = ps_hi.tile([65, group, P], F16)
                for j in range(group):
                    k = g0 + j
                    nc.tensor.transpose(comb_lo[:, j, :], xh[:, k, 0:128],
                                        ident)
                    nc.tensor.transpose(comb_hi[:, j, :], xh[:, k, 128:XW],
                                        ident)
                sb = sbT.tile([128, group, 2 * P], F16)
                nc.vector.tensor_copy(out=sb[:, :, 0:P], in_=comb_lo)
                nc.scalar.copy(out=sb[0:65, :, P:2 * P], in_=comb_hi)

                for q in range(group // quad):
                    k0 = g0 + q * quad
                    z4 = ps_z.tile([P, quad, 256], F32)
                    for j in range(quad):
                        nc.tensor.matmul(z4[:, j, 0:DIM],
                                         sb[:, q * quad + j, 0:P], wlo,
                                         start=True, stop=False)
                        nc.tensor.matmul(z4[:, j, 0:DIM],
                                         sb[0:65, q * quad + j, P:2 * P],
                                         whi, start=False, stop=True)
                    # plain batched f16->f32 output copy, alternating engine
                    if (g + q) % 2 == 0:
                        nc.vector.tensor_copy(out=oc[:, k0:k0 + quad, :],
                                              in_=z4[:, :, 0:DIM])
                    else:
                        nc.scalar.copy(out=oc[:, k0:k0 + quad, :],
                                       in_=z4[:, :, 0:DIM])
            # store halves as they complete so the tail drains early
            nc.sync.dma_start(out=o3[:, a0:a0 + chunk_tiles // 2, :],
                              in_=oc[:, 0:chunk_tiles // 2, :])
            nc.sync.dma_start(out=o3[:, a0 + chunk_tiles // 2:
                                     a0 + chunk_tiles, :],
                              in_=oc[:, chunk_tiles // 2:chunk_tiles, :])

        # software pipeline: stats/cast of chunk n+1 overlaps the
        # transpose/matmul/output phase of chunk n
        xh_prev = front(0)
        for n in range(1, nchunks):
            xh_next = front(n)
            back(n - 1, xh_prev)
            xh_prev = xh_next
        back(nchunks - 1, xh_prev)

    nc.compile()
    return nc


def _host_weights(inputs):
    out_w = np.asarray(inputs["out_w"], np.float32)
    out_b = np.asarray(inputs["out_b"], np.float32)
    ln_w = np.asarray(inputs["ln_w"], np.float32)
    ln_b = np.asarray(inputs["ln_b"], np.float32)
    W = (out_w * ln_w[None, :]).T.astype(np.float32)   # [ch_in, ch_out]
    c = ln_b @ out_w.T + out_b
    assert np.abs(c).max() < 1e-6, "nonzero projection bias not supported"
    # device computes rstd' = rstd/sqrt(192) (from n*var); fold sqrt(192) here
    Wd = np.sqrt(np.float32(DIM)) * W
    s = Wd.sum(axis=0)
    wt_lo = Wd[0:128].astype(np.float16)
    wt_hi = np.concatenate([Wd[128:192], -s[None, :]], axis=0)
    return wt_lo, wt_hi.astype(np.float16), W


def _expected_sample(x, W, idx):
    """Host-side reference for a token subset (for the cheap self-check)."""
    xs = x[idx].astype(np.float32)
    mu = xs.mean(-1, keepdims=True)
    var = xs.var(-1, keepdims=True)
    xn = (xs - mu) / np.sqrt(var + np.float32(EPS_EFF))
    return xn @ W


_NC_CACHE = {}


def _run(nc, in_maps):
    from concourse.bass_utils import run_bass_kernel_spmd
    res = run_bass_kernel_spmd(nc, in_maps, core_ids=list(range(NCORES)))
    return np.concatenate(
        [res.results[i]["out_shard"] for i in range(NCORES)], axis=0)


def kernel(**inputs):
    x = np.ascontiguousarray(np.asarray(inputs["x"], np.float32).reshape(L, DIM))
    wt_lo, wt_hi, W = _host_weights(inputs)
    if "nc" not in _NC_CACHE:
        _NC_CACHE["nc"] = build_nc()
    nc = _NC_CACHE["nc"]
    in_maps = [
        {"x_shard": x[i * SHARD:(i + 1) * SHARD], "wt_lo": wt_lo,
         "wt_hi": wt_hi}
        for i in range(NCORES)
    ]
    out = _run(nc, in_maps)
    # Cheap sanity check on a random token subset; one retry guards against
    # rare transient device glitches on a cold first execution.
    idx = np.random.default_rng(0).choice(L, 512, replace=False)
    want = _expected_sample(x, W, idx)
    err = np.abs(out[idx] - want).max() / max(np.abs(want).max(), 1e-6)
    if not np.isfinite(err) or err > 5e-3:
        out = _run(nc, in_maps)
    return out.reshape(1, L, DIM)


# revision 23
# speedup vs baseline: 1.0426x; 1.0426x over previous
"""Trainium2 Bass kernel for nn_ASSM_Illumination (B=1, L=65536, DIM=192, 8 cores).

Mathematical reduction
----------------------
The reference computes: convs -> routing MLP -> gumbel one-hot -> sort by
illumination key -> gated selective scan -> LayerNorm -> projection -> unsort.

The scan output is y[l] = (h_l @ A_log) + xs[l], where (h_l @ A_log) is a
per-token *scalar* broadcast over channels, and xs[l] = gamma_l * x[l] + beta_l
with per-token scalars gamma_l, beta_l.  The LayerNorm over channels is
invariant to per-token additive shifts, so the scan scalar and beta cancel
exactly; gamma cancels except through the eps term:

    LN(y)[l] = (x_l - mean(x_l)) / sqrt(var(x_l) + eps/gamma_l^2)

gamma_l = 0.3 + 0.7*sigmoid(key_l) in [0.65, 0.81], and with eps = 1e-5 the
output's sensitivity to gamma is ~1e-5 relative, far below the reference's own
fp32 noise floor.  The sort + unsort is a permutation and its inverse applied
around per-token ops: identity.

So the kernel computes, per token:
    out[l] = ((x_l - mu_l) * rstd_l) @ W + c
with W = (out_w * ln_w).T, c = ln_b @ out_w.T + out_b (c == 0 for this
problem's zero biases), rstd_l = 1/sqrt(var(x_l) + 1e-5/g0^2), g0 = 0.735.

Device dataflow (v5) -- per 128-token tile, fp16 matmul path:
  (x - mu)*rstd @ W = (rstd*x) @ W - (rstd*mu) * s   with s = colsum(W):
  - DVE:  paired-tile bn_stats on raw f32 x: two tiles streamed
          channel-interleaved (p a c -> p c a) land on the even/odd element
          stats of ONE bn_stats -> exact (mean, n*var) per tile, no bn_aggr
  - ACT:  rstd' = 1/sqrt(n*var + n*eps) batched (sqrt(n) folded into host
          weights); DVE reciprocal
  - DVE:  raw mu written into column 192 of the f32 input tile, so the
          per-tile cast+scale (xh = rstd * x, f32->f16, split DVE/ACT)
          produces the q = mu*rstd column for free
  - PE:   transpose [tok, 0:128] and [tok, 128:193], 8 tiles per full
          PSUM bank; one PSUM->SBUF copy per bank (DVE lo / ACT hi)
  - PE:   2 matmuls vs weights [W_lo] and [W_hi; -s] into f16 PSUM
  - DVE/ACT: plain 4-tile-batched f16->f32 output copies (scale already
          applied on the input side), alternating engines
  - DMA:  16-tile chunks; loads on SP hwdge, stores on Pool SWDGE

Sharding: L=65536 tokens split contiguously across 8 cores (8192 each); the
tiny weight matrix is replicated.  No collectives.  Token t of a shard lives
at partition p = t // 64, slot a = t % 64 so DMAs move long contiguous lines
per partition.
"""

import numpy as np
from contextlib import ExitStack

import concourse.bass as bass
import concourse.bacc as bacc
import concourse.tile as tile
from concourse import mybir
from concourse.masks import make_identity

L = 65536
DIM = 192
NCORES = 8
SHARD = L // NCORES          # 8192 tokens per core
P = 128                      # tokens per tile (partition dim)
G0 = 0.735                   # mid-range gamma; output sensitivity to g0 is ~1e-5
EPS_EFF = 1e-5 / (G0 * G0)
XW = DIM + 1                 # row: 192 channels + mu (f32) / q (f16)

F32 = mybir.dt.float32
F16 = mybir.dt.float16
AF = mybir.ActivationFunctionType
ALU = mybir.AluOpType


def bn_stats_raw(eng, out, in_):
    """Raw InstBNStats: bass's wrapper would treat the 3D input as groups,
    but the hardware streams the AP flat and computes separate even/odd
    element stats -- exactly the 2-tile interleave we want."""
    return eng.add_instruction(mybir.InstBNStats(
        name=eng.bass.get_next_instruction_name(),
        ins=[eng.lower_ap(in_)],
        outs=[eng.lower_ap(out)]))


def build_nc(shard=SHARD, chunk_tiles=16, group=8, quad=4):
    """One-core program; run SPMD on 8 cores with different x shards."""
    ntiles = shard // P          # 64
    nchunks = ntiles // chunk_tiles
    nc = bacc.Bacc("TRN2", target_bir_lowering=False, debug=False,
                   num_devices=NCORES)

    x_d = nc.dram_tensor("x_shard", (shard, DIM), F32, kind="ExternalInput")
    # wt_lo: sqrt(192)*W[0:128]  (f16)
    wlo_d = nc.dram_tensor("wt_lo", (128, DIM), F16, kind="ExternalInput")
    # wt_hi rows: [sqrt(192)*W[128:192]; -colsum(sqrt(192)*W)] -> [65, 192]
    whi_d = nc.dram_tensor("wt_hi", (65, DIM), F16, kind="ExternalInput")
    o_d = nc.dram_tensor("out_shard", (shard, DIM), F32, kind="ExternalOutput")

    # token t = p * (shard/128) + a  lives at partition p, slot a
    x3 = x_d[:, :].rearrange("(p a) c -> p a c", p=P)
    o3 = o_d[:, :].rearrange("(p a) c -> p a c", p=P)

    with tile.TileContext(nc) as tc, ExitStack() as ctx:
        singles = ctx.enter_context(tc.tile_pool(name="singles", bufs=1))
        xin = ctx.enter_context(tc.tile_pool(name="xin", bufs=3))
        xh_pool = ctx.enter_context(tc.tile_pool(name="xh", bufs=3))
        xout = ctx.enter_context(tc.tile_pool(name="xout", bufs=3))
        stats = ctx.enter_context(tc.tile_pool(name="stats", bufs=4))
        sbT = ctx.enter_context(tc.tile_pool(name="sbT", bufs=3))
        ps_lo = ctx.enter_context(
            tc.tile_pool(name="ps_lo", bufs=2, space=bass.MemorySpace.PSUM))
        ps_hi = ctx.enter_context(
            tc.tile_pool(name="ps_hi", bufs=2, space=bass.MemorySpace.PSUM))
        ps_z = ctx.enter_context(
            tc.tile_pool(name="ps_z", bufs=2, space=bass.MemorySpace.PSUM))

        ident = singles.tile([P, P], F16)
        make_identity(nc, ident)
        eps_t = singles.tile([P, 1], F32)
        nc.vector.memset(eps_t, float(DIM * EPS_EFF))
        # weights ride the ACT hwdge queue so SP's first trigger is the
        # first x-chunk load
        wlo = singles.tile([128, DIM], F16)
        nc.scalar.dma_start(out=wlo, in_=wlo_d[:, :])
        whi = singles.tile([65, DIM], F16)
        nc.scalar.dma_start(out=whi, in_=whi_d[:, :])

        ngroups = chunk_tiles // group

        def front(n):
            """load + stats + cast for chunk n; returns the f16 tile."""
            a0 = n * chunk_tiles
            xc = xin.tile([P, chunk_tiles, XW], F32)
            if n == 0:
                # split the first load so the stats pipeline starts sooner
                for (b0, b1) in ((0, 2), (2, 4), (4, 8), (8, chunk_tiles)):
                    nc.sync.dma_start(out=xc[:, b0:b1, 0:DIM],
                                      in_=x3[:, a0 + b0:a0 + b1, :])
            else:
                nc.sync.dma_start(out=xc[:, :, 0:DIM],
                                  in_=x3[:, a0:a0 + chunk_tiles, :])

            st = stats.tile([P, chunk_tiles // 2, 6], F32, tag="st")
            stdv = stats.tile([P, chunk_tiles], F32, tag="stdv")
            rstd = stats.tile([P, chunk_tiles], F32, tag="rstd")
            xh = xh_pool.tile([P, chunk_tiles, XW], F16)
            H = chunk_tiles // 2
            for h in range(2):
                h0 = h * H
                # DVE: paired-tile stats (even/odd split of one bn_stats)
                for k in range(h0, h0 + H, 2):
                    pair = xc[:, k:k + 2, 0:DIM].rearrange("p a c -> p c a")
                    bn_stats_raw(nc.vector, st[:, k // 2, :], pair)
                # rstd' = 1/sqrt(n*var + n*eps); raw mu -> xc col 192
                sh = st[:, h0 // 2:(h0 + H) // 2, :]
                nv = sh[:, :, 2:6:3]   # strided view of n*var
                mu = sh[:, :, 1:5:3]   # strided view of means
                nc.scalar.activation(out=stdv[:, h0:h0 + H], in_=nv,
                                     func=AF.Sqrt, bias=eps_t)
                nc.vector.reciprocal(out=rstd[:, h0:h0 + H],
                                     in_=stdv[:, h0:h0 + H])
                nc.scalar.copy(out=xc[:, h0:h0 + H, DIM], in_=mu)
                # cast+scale for the whole half in ONE tensor_tensor:
                # xh = rstd * [x | mu] (f16) with rstd broadcast over the
                # 193 row elements via a stride-0 AP dim; col 192 becomes q
                r = rstd[:, h0:h0 + H]
                rb = bass.AP(r.tensor, r.offset, list(r.ap) + [[0, XW]])
                nc.vector.tensor_tensor(out=xh[:, h0:h0 + H, :],
                                        in0=xc[:, h0:h0 + H, :],
                                        in1=rb, op=ALU.mult)
            return xh

        def back(n, xh):
            """transpose + matmul + output for chunk n."""
            a0 = n * chunk_tiles
            oc = xout.tile([P, chunk_tiles, DIM], F32)
            for g in range(ngroups):
                g0 = g * group
                # PE transposes into full-bank PSUM tiles (8 tiles per bank)
                comb_lo = ps_lo.tile([128, group, P], F16)
                comb_hi = ps_hi.tile([65, group, P], F16)
                for j in range(group):
                    k = g0 + j
                    nc.tensor.transpose(comb_lo[:, j, :], xh[:, k, 0:128],
                                        ident)
                    nc.tensor.transpose(comb_hi[:, j, :], xh[:, k, 128:XW],
                                        ident)
                sb = sbT.tile([128, group, 2 * P], F16)
                nc.scalar.copy(out=sb[:, :, 0:P], in_=comb_lo)
                nc.scalar.copy(out=sb[0:65, :, P:2 * P], in_=comb_hi)

                for q in range(group // quad):
                    k0 = g0 + q * quad
                    z4 = ps_z.tile([P, quad, 256], F32)
                    for j in range(quad):
                        nc.tensor.matmul(z4[:, j, 0:DIM],
                                         sb[:, q * quad + j, 0:P], wlo,
                                         start=True, stop=False)
                        nc.tensor.matmul(z4[:, j, 0:DIM],
                                         sb[0:65, q * quad + j, P:2 * P],
                                         whi, start=False, stop=True)
                    # plain batched output copy (scale already on input side)
                    nc.scalar.copy(out=oc[:, k0:k0 + quad, :],
                                   in_=z4[:, :, 0:DIM])
            # store halves as they complete so the tail drains early
            nc.sync.dma_start(out=o3[:, a0:a0 + chunk_tiles // 2, :],
                              in_=oc[:, 0:chunk_tiles // 2, :])
            nc.sync.dma_start(out=o3[:, a0 + chunk_tiles // 2:
                                     a0 + chunk_tiles, :],
                              in_=oc[:, chunk_tiles // 2:chunk_tiles, :])

        # software pipeline: stats/cast of chunk n+1 overlaps the
        # transpose/matmul/output phase of chunk n
        xh_prev = front(0)
        for n in range(1, nchunks):
            xh_next = front(n)
            back(n - 1, xh_prev)
            xh_prev = xh_next
        back(nchunks - 1, xh_prev)

    nc.compile()
    return nc


def _host_weights(inputs):
    out_w = np.asarray(inputs["out_w"], np.float32)
    out_b = np.asarray(inputs["out_b"], np.float32)
    ln_w = np.asarray(inputs["ln_w"], np.float32)
    ln_b = np.asarray(inputs["ln_b"], np.float32)
    W = (out_w * ln_w[None, :]).T.astype(np.float32)   # [ch_in, ch_out]
    c = ln_b @ out_w.T + out_b
    assert np.abs(c).max() < 1e-6, "nonzero projection bias not supported"
    # device computes rstd' = rstd/sqrt(192) (from n*var); fold sqrt(192) here
    Wd = np.sqrt(np.float32(DIM)) * W
    s = Wd.sum(axis=0)
    wt_lo = Wd[0:128].astype(np.float16)
    wt_hi = np.concatenate([Wd[128:192], -s[None, :]], axis=0)
    return wt_lo, wt_hi.astype(np.float16), W


def _expected_sample(x, W, idx):
    """Host-side reference for a token subset (for the cheap self-check)."""
    xs = x[idx].astype(np.float32)
    mu = xs.mean(-1, keepdims=True)
    var = xs.var(-1, keepdims=True)
    xn = (xs - mu) / np.sqrt(var + np.float32(EPS_EFF))
    return xn @ W


_NC_CACHE = {}


def _run(nc, in_maps):
    from concourse.bass_utils import run_bass_kernel_spmd
    res = run_bass_kernel_spmd(nc, in_maps, core_ids=list(range(NCORES)))
    return np.concatenate(
        [res.results[i]["out_shard"] for i in range(NCORES)], axis=0)


def kernel(**inputs):
    x = np.ascontiguousarray(np.asarray(inputs["x"], np.float32).reshape(L, DIM))
    wt_lo, wt_hi, W = _host_weights(inputs)
    if "nc" not in _NC_CACHE:
        _NC_CACHE["nc"] = build_nc()
    nc = _NC_CACHE["nc"]
    in_maps = [
        {"x_shard": x[i * SHARD:(i + 1) * SHARD], "wt_lo": wt_lo,
         "wt_hi": wt_hi}
        for i in range(NCORES)
    ]
    out = _run(nc, in_maps)
    # Cheap sanity check on a random token subset; one retry guards against
    # rare transient device glitches on a cold first execution.
    idx = np.random.default_rng(0).choice(L, 512, replace=False)
    want = _expected_sample(x, W, idx)
    err = np.abs(out[idx] - want).max() / max(np.abs(want).max(), 1e-6)
    if not np.isfinite(err) or err > 5e-3:
        out = _run(nc, in_maps)
    return out.reshape(1, L, DIM)
